# revision 48
# baseline (speedup 1.0000x reference)
"""FP8 GEMM kernel for Trainium2 (8 NeuronCores, SPMD data-parallel over tokens).

Computes: out = fp16( fp32( e5m2(x) @ e4m3(weight.T) ) + bias )
  x      [4, 4096, 4096] fp16
  weight [4096, 4096]    fp16  (out_features, in_features)
  bias   [4096]          fp16
  out    [4, 4096, 4096] fp16

Sharding: token dim (B*S = 16384) split across 8 cores (2048 rows each);
weight + bias replicated. No collectives; host concatenates the outputs.

The host quantizes both operands to fp8 (ml_dtypes RNE — bit-identical to
the reference's own jnp casts) and pre-packs them into per-tile K-major
blocks (`[tile][ki=128][ko=32][free]`), so every device load is a plain
contiguous fp8 HWDGE DMA — no in-flight cast, half the bytes of an fp16
stream.  The bias is pre-broadcast on host to [128, 4096] so the device
load is one plain 1MB DMA instead of a slow replicating DMA.

Per-core kernel, 461.9-462.8us measured vs a ~460us practical floor:
  2048 MMs x 215.8ns issue (442.0us, the fp8 DoubleRow stream rate:
  512 cols @ 2.4GHz + ~2.5ns NX) + ~2us residual ramp drift + ~13us
  cold-start (preamble ~7us + first 160KB on a cold ~40GB/s DMA) +
  ~11.3us post-MM (final evict/store/barrier ~4.2us + fixed runtime
  teardown ~7us, both inside the measured window; the preamble is
  excluded).  NOTE: the profiler drops one MATMUL record every
  10.792us (~41/run), which shows up as fake periodic 432ns "gaps" in
  the trace — they are NOT stalls (span arithmetic proves it); do not
  optimize against them.

Design (each point earned by a trace-diagnosed failure):
 - DoubleRow fp8 matmuls (K=256/instr, N=512 moving) accumulate fp32
   into PSUM; x (8MB fp8) stays resident, w n-tiles stream through a
   3-deep pool.  LDWEIGHTS (135ns) hides behind the 216ns MM stream.
 - DMA model (measured): a queue's transfers fan out as ~4KB packets
   over 16 DMA engines (~21GB/s each, ~330-380GB/s aggregate), but a
   consumer matmul gates on the completion semaphore of the WHOLE
   transfer that wrote its region -> coarse multi-chunk transfers
   cause ~3us PE stalls.  Each push instruction costs ~0.65us of queue
   sequencer time -> per-chunk granularity everywhere halves effective
   bandwidth.  Ramp uses singles for kc0..5, pairs after (23 pushes).
 - ALL ramp-critical data (w0 + a combined ksub-major x block for
   m=0..6, fully-contiguous 7KB runs — an m-major layout with 1KB runs
   measurably crawls) rides the sync queue EXCLUSIVELY in consumption
   order.  Bulk loads (x7.., bias, w1..w7) sit on scalar, each gated by
   a tiny ACT-copy write into its destination sourced from a kc13 ramp
   region: the WAW dep is the only thing the compile-time Tile
   scheduler cannot hoist bulk pushes around (it DOES hoist dep-free
   instructions past a single dep-carrying gate).  Stores ride sync
   (idle after the ramp) so no load waits behind a store.
 - The PE clock starts gated at 1.2GHz and un-throttles after ~3.4us
   of sustained busy; ~2us of idle re-gates it (~1.7us cold rerun).
   28 dummy N=128 matmuls on a zeroed scratch tile start the activity
   window at the preamble end, and FILL dummies woven between early
   warm-up chunks absorb the cold-DMA gaps, so the clock reaches
   2.4GHz by ~12us and never re-throttles.
 - Warm-up interleaves m=0..6 k-chunk-outer across 7 PSUM banks (the
   8th holds the dummies), so each arriving 352KB k-chunk unlocks 7
   matmuls (1.51us) — slower than the exclusive queue delivers.
 - Bias comes pre-broadcast from host ([128,4096], one plain 1MB DMA;
   a replicating DMA costs ~8-10us of DMA-engine time).  Bias add is
   fused into the PSUM eviction on DVE (its only job).  The final
   group's eviction is split into 2x256-col strips stored on
   scalar+sync to overlap the closing HBM-write receipt.
 - Sub-region (N=256) psum starts inside one accumulation group
   mis-compute on HW (rel err 0.24) — do not re-attempt without
   understanding why.
"""

import sys

if "/opt/trn_rl_repo" not in sys.path:
    sys.path.insert(0, "/opt/trn_rl_repo")

import ml_dtypes
import numpy as np

B, S, DIN, DOUT = 4, 4096, 4096, 4096
NCORES = 8
M_TOTAL = B * S              # 16384
M_LOC = M_TOTAL // NCORES    # 2048
P = 128
M_TILES = M_LOC // P         # 16 m-tiles of 128 rows
N_TILE = 512
N_TILES = DOUT // N_TILE     # 8
K_SUB = DIN // P             # 32 k-subtiles of 128
K_CHUNKS = K_SUB // 2        # 16 DoubleRow chunks of 256
WARM_M = 7                   # m-groups interleaved during the w0 ramp
N_DUMMY = 36                 # HAM-warming dummy matmuls (N=128): they end
                             # ~11.2us, still before the earliest observed
                             # first-chunk arrival (12.0us), and on a
                             # slow-cold-DMA run (arrival up to 14.4us
                             # observed) they shrink the PE-idle window
                             # below the ~3.4us clock re-throttle threshold

_cached_nc = None


def _build():
    global _cached_nc
    if _cached_nc is not None:
        return _cached_nc

    import concourse.mybir as mybir
    import concourse.tile as tile
    from concourse import bacc

    nc = bacc.Bacc("TRN2", target_bir_lowering=False, debug=False,
                   num_devices=NCORES)

    # host-packed fp8 K-major tile blocks (see make_in_maps)
    xd01 = nc.dram_tensor("xd01", [P, K_SUB, WARM_M, P], mybir.dt.float8e5,
                          kind="ExternalInput")
    xd = nc.dram_tensor("xd", [M_TILES - WARM_M, P, K_SUB, P],
                        mybir.dt.float8e5, kind="ExternalInput")
    wd = nc.dram_tensor("wd", [N_TILES, P, K_SUB, N_TILE], mybir.dt.float8e4,
                        kind="ExternalInput")
    brep = nc.dram_tensor("brep", [P, DOUT], mybir.dt.float16,
                          kind="ExternalInput")
    out = nc.dram_tensor("out", [M_LOC, DOUT], mybir.dt.float16,
                         kind="ExternalOutput")

    with tile.TileContext(nc) as tc:
        with tc.tile_pool(name="w8p", bufs=3) as w8p, \
             tc.tile_pool(name="x8p", bufs=1) as x8p, \
             tc.tile_pool(name="outp", bufs=8) as outp, \
             tc.tile_pool(name="cst", bufs=1) as cst, \
             tc.tile_pool(name="psum", bufs=8, space="PSUM") as psump:

            # resident fp8 x: m=0..6 in one combined ksub-MAJOR block (so
            # one ramp DMA delivers a k-slice for all warm groups at once,
            # AND the transfer is fully contiguous — 7KB runs/partition;
            # the old m-major layout had 1KB runs and its packets crawled),
            # the rest as per-m tiles
            x01 = x8p.tile([P, K_SUB, WARM_M, P], mybir.dt.float8e5,
                           tag="x01", name="x01")
            x8 = {m: x8p.tile([P, K_SUB, P], mybir.dt.float8e5,
                              tag=f"x8_{m}", name=f"x8_{m}")
                  for m in range(WARM_M, M_TILES)}

            def xap(m, kc):
                if m < WARM_M:
                    return x01[:, 2 * kc:2 * kc + 2, m, :]
                return x8[m][:, 2 * kc:2 * kc + 2, :]

            w8 = {}

            # Each bulk load's destination gets a tiny ACT-copy write
            # sourced from a LATE ramp chunk of x01 (kc13): the DMA then
            # carries a WAW dependency on the gate, which the compile-time
            # Tile scheduler cannot hoist around (v6 measured dep-free
            # scalar pushes hoisted ahead of a single gating copy,
            # reintroducing the 2x ramp slowdown).
            def gate(dst2):
                nc.scalar.copy(dst2, x01[0:1, 27, WARM_M - 1, 126:128])

            def load_w(j):
                w8[j] = w8p.tile([P, K_SUB, N_TILE], mybir.dt.float8e4,
                                 tag="w8", name=f"w8_{j}")
                gate(w8[j][0:1, 31, 510:512])
                nc.scalar.dma_start(w8[j][:], wd[j, :, :, :])

            # ---- sync-queue program: strict priority order.  Emission
            # order = per-queue FIFO order; the HWDGE ring pops descriptors
            # in order, so data ARRIVES in consumption order.  Nothing else
            # rides sync until the tail, so the ramp gets the full ~350GB/s.
            # Step granularity balances two measured failure modes: a
            # chunk's matmuls gate on the completion semaphore of the
            # whole transfer that wrote it (coarse steps -> ~3us stalls),
            # while per-chunk steps double the push count and halve the
            # queue's effective bandwidth (~0.65us/push sequencer cost).
            # Singles for kc0..3 (cold window), pairs after.
            RAMP = [(2 * k, 2 * k + 2) for k in range(6)] + \
                   [(12 + 4 * s, 16 + 4 * s) for s in range(5)]
            w8[0] = w8p.tile([P, K_SUB, N_TILE], mybir.dt.float8e4,
                             tag="w8", name="w8_0")
            # A single serial sync stream beats splitting the ramp across
            # sync+scalar: the parallel variants start ~1us earlier (the
            # cold-DMA limit is per-queue) but the two queues don't
            # co-pace reliably and ~1us chunk-lag stalls eat the gain
            # (measured 462.9/464.2 vs 461.9us serial).
            first = True
            for a, b in RAMP:
                nc.sync.dma_start(w8[0][:, a:b, :], wd[0, :, a:b, :])
                if first:
                    # split the first x chunk so MM(m=0,kc=0) unblocks on
                    # 160KB instead of 352KB (cold DMA is slow)
                    nc.sync.dma_start(x01[:, a:b, 0:1, :],
                                      xd01[:, a:b, 0:1, :])
                    nc.sync.dma_start(x01[:, a:b, 1:WARM_M, :],
                                      xd01[:, a:b, 1:WARM_M, :])
                    first = False
                else:
                    nc.sync.dma_start(x01[:, a:b, :, :], xd01[:, a:b, :, :])

            # ---- scalar-queue program: all slack-tolerant bulk, each
            # transfer gated behind the ramp (see gate() above) so none of
            # it competes with the ramp for DMA bandwidth (v5 measured the
            # shared-queue ramp at half rate).  Stores ride sync (free
            # after the ramp) — never behind a pool-anti-dep-blocked push.
            gate(x8[WARM_M][0:1, 31, 126:128])
            nc.scalar.dma_start(x8[WARM_M][:], xd[0, :, :, :])
            bias_rep = cst.tile([P, DOUT], mybir.dt.float16)
            gate(bias_rep[0:1, 0:2])
            nc.scalar.dma_start(bias_rep[:], brep.ap())
            for m in range(WARM_M + 1, M_TILES):
                gate(x8[m][0:1, 31, 126:128])
                nc.scalar.dma_start(x8[m][:], xd[m - WARM_M, :, :, :])
            for j in range(1, N_TILES):
                # w3.. pushes wait on the 3-deep pool's anti-dep (column
                # j-3 finished) — head-of-line blocking is fine, nothing
                # urgent behind them on scalar
                load_w(j)

            # ---- HAM warm-up: dummy matmuls on a zeroed scratch tile keep
            # the PE activity window busy from the preamble end until the
            # first real chunk lands (~9.5us), so the 2.4GHz un-throttle
            # fires at ~10.7us instead of ~16.5us.
            dum = cst.tile([P, 2, P], mybir.dt.float8e5, name="dum")
            nc.vector.memset(dum[:], 0)
            psum = {}
            dps = psump.tile([P, P], mybir.dt.float32, tag="ps", name="ps_dum")

            def dummies(n):
                for _ in range(n):
                    nc.tensor.matmul(
                        dps[:], dum[:], dum[:], start=True, stop=True,
                        perf_mode=mybir.MatmulPerfMode.DoubleRow,
                    )

            dummies(N_DUMMY)

            def mm(j, m, kc):
                nc.tensor.matmul(
                    psum[m][:],
                    xap(m, kc),
                    w8[j][:, 2 * kc:2 * kc + 2, :],
                    start=(kc == 0),
                    stop=(kc == K_CHUNKS - 1),
                    perf_mode=mybir.MatmulPerfMode.DoubleRow,
                )

            def evict(j, m, split=False):
                if not split:
                    ob = outp.tile([P, N_TILE], mybir.dt.float16, tag="ob",
                                   name=f"ob_{j}_{m}")
                    nc.vector.tensor_add(
                        ob[:], psum[m][:],
                        bias_rep[:, j * N_TILE:(j + 1) * N_TILE])
                    nc.sync.dma_start(
                        out[m * P:(m + 1) * P,
                            j * N_TILE:(j + 1) * N_TILE], ob[:])
                    return
                # final group: halve the eviction and alternate the stores
                # across both HWDGE queues so the closing HBM-write receipt
                # overlaps the last DVE strip (4 even strips and a 384/128
                # asymmetric split both measured worse: the strips
                # serialize at ~325ns each on the DVE)
                h = N_TILE // 2
                for c in range(2):
                    eng = nc.scalar if c % 2 == 0 else nc.sync
                    ob = outp.tile([P, h], mybir.dt.float16, tag="obs",
                                   name=f"ob_{j}_{m}_{c}")
                    nc.vector.tensor_add(
                        ob[:], psum[m][:, c * h:(c + 1) * h],
                        bias_rep[:, j * N_TILE + c * h:
                                 j * N_TILE + (c + 1) * h])
                    eng.dma_start(
                        out[m * P:(m + 1) * P,
                            j * N_TILE + c * h:j * N_TILE + (c + 1) * h],
                        ob[:])

            def do_group(j, m):
                psum[m] = psump.tile([P, N_TILE], mybir.dt.float32, tag="ps",
                                     name=f"ps_{j}_{m}")
                for kc in range(K_CHUNKS):
                    mm(j, m, kc)
                evict(j, m,
                      split=(j == N_TILES - 1 and m == M_TILES - 1))

            # ---- warm-up: column 0, m=0..6 k-chunk-outer so each arriving
            # w0/x chunk unlocks WARM_M matmuls (PE consumes a 352KB chunk
            # in 1.5us warm — slower than the exclusive sync queue delivers).
            # Dummy fill between the early chunks absorbs the cold-DMA wall
            # (~1MB by 14us) without letting the PE idle long enough to
            # re-gate the clock.
            # FILL sizes target the residual 300-500ns data waits observed
            # at the kc0-rest/kc1/kc2 boundaries in EVERY sampled run (the
            # PE always outpaces the cold DMA there, so the dummies are
            # free in practice)
            FILL = {0: 20, 1: 8, 2: 5, 3: 3, 4: 2, 5: 3, 6: 2, 7: 1, 8: 2}
            for m in range(WARM_M):
                psum[m] = psump.tile([P, N_TILE], mybir.dt.float32, tag="ps",
                                     name=f"ps_0_{m}")
            for kc in range(K_CHUNKS):
                for m in range(WARM_M):
                    mm(0, m, kc)
                    if kc == 0 and m == 0:
                        dummies(FILL[0])
                dummies(FILL.get(kc + 1, 0))
            # bias_rep lands ~27us, warm-up ends ~38us: plain fused
            # evictions work (no decoupled copy needed)
            for m in range(WARM_M):
                evict(0, m)

            # ---- steady state: column-major, group-serial; w tiles were
            # all queued upfront, paced by the pool anti-deps ----
            for m in range(WARM_M, M_TILES):
                do_group(0, m)
            for j in range(1, N_TILES):
                for m in range(M_TILES):
                    do_group(j, m)

    nc.compile()
    _cached_nc = nc
    return nc


def make_in_maps(x, weight, bias):
    x = np.asarray(x)
    weight = np.asarray(weight)
    bias = np.ascontiguousarray(np.asarray(bias))
    assert x.dtype == np.float16 and weight.dtype == np.float16

    # quantize exactly as the reference does (RNE casts)
    x8 = x.astype(ml_dtypes.float8_e5m2)
    w8 = weight.astype(ml_dtypes.float8_e4m3fn)

    # weight [DOUT, DIN] -> [j, ki, ko, n]: wd[j,ki,ko,n] = w8[j*512+n,
    # ko*128+ki] (i.e. weight.T in per-tile K-major blocks)
    wd = np.ascontiguousarray(
        w8.reshape(N_TILES, N_TILE, K_SUB, P).transpose(0, 3, 2, 1))

    # bias pre-broadcast to all 128 partitions: one plain contiguous DMA
    brep = np.ascontiguousarray(np.broadcast_to(bias, (P, DOUT)))

    xf = x8.reshape(M_TOTAL, DIN)
    in_maps = []
    for c in range(NCORES):
        xc = xf[c * M_LOC:(c + 1) * M_LOC]
        # [M_LOC, DIN] -> [m-tile, ki, ko, m]: xd[t,ki,ko,m] = xc[t*128+m,
        # ko*128+ki]
        xdt = np.ascontiguousarray(
            xc.reshape(M_TILES, P, K_SUB, P).transpose(0, 3, 2, 1))
        # first WARM_M m-tiles also packed as one [ki, ko, t, m] block so
        # each ramp DMA delivers a k-slice for all warm groups at once as
        # ONE fully-contiguous transfer
        xd01 = np.ascontiguousarray(xdt[:WARM_M].transpose(1, 2, 0, 3))
        in_maps.append({"xd01": xd01, "xd": np.ascontiguousarray(xdt[WARM_M:]),
                        "wd": wd, "brep": brep})
    return in_maps


def gather_out(results):
    out = np.concatenate([r["out"] for r in results], axis=0)
    return out.reshape(B, S, DOUT)


def kernel(x, weight, bias):
    from concourse.bass_utils import run_bass_kernel_spmd

    nc = _build()
    in_maps = make_in_maps(x, weight, bias)
    res = run_bass_kernel_spmd(nc, in_maps, core_ids=list(range(NCORES)))
    return gather_out(res.results)


# revision 49
# speedup vs baseline: 1.0010x; 1.0010x over previous
"""FP8 GEMM kernel for Trainium2 (8 NeuronCores, SPMD data-parallel over tokens).

Computes: out = fp16( fp32( e5m2(x) @ e4m3(weight.T) ) + bias )
  x      [4, 4096, 4096] fp16
  weight [4096, 4096]    fp16  (out_features, in_features)
  bias   [4096]          fp16
  out    [4, 4096, 4096] fp16

Sharding: token dim (B*S = 16384) split across 8 cores (2048 rows each);
weight + bias replicated. No collectives; host concatenates the outputs.

The host quantizes both operands to fp8 (ml_dtypes RNE — bit-identical to
the reference's own jnp casts) and pre-packs them into per-tile K-major
blocks (`[tile][ki=128][ko=32][free]`), so every device load is a plain
contiguous fp8 HWDGE DMA — no in-flight cast, half the bytes of an fp16
stream.  The bias is pre-broadcast on host to [128, 4096] so the device
load is one plain 1MB DMA instead of a slow replicating DMA.

Per-core kernel, 461.9-462.8us measured vs a ~460us practical floor:
  2048 MMs x 215.8ns issue (442.0us, the fp8 DoubleRow stream rate:
  512 cols @ 2.4GHz + ~2.5ns NX) + ~2us residual ramp drift + ~13us
  cold-start (preamble ~7us + first 160KB on a cold ~40GB/s DMA) +
  ~11.3us post-MM (final evict/store/barrier ~4.2us + fixed runtime
  teardown ~7us, both inside the measured window; the preamble is
  excluded).  NOTE: the profiler drops one MATMUL record every
  10.792us (~41/run), which shows up as fake periodic 432ns "gaps" in
  the trace — they are NOT stalls (span arithmetic proves it); do not
  optimize against them.

Design (each point earned by a trace-diagnosed failure):
 - DoubleRow fp8 matmuls (K=256/instr, N=512 moving) accumulate fp32
   into PSUM; x (8MB fp8) stays resident, w n-tiles stream through a
   3-deep pool.  LDWEIGHTS (135ns) hides behind the 216ns MM stream.
 - DMA model (measured): a queue's transfers fan out as ~4KB packets
   over 16 DMA engines (~21GB/s each, ~330-380GB/s aggregate), but a
   consumer matmul gates on the completion semaphore of the WHOLE
   transfer that wrote its region -> coarse multi-chunk transfers
   cause ~3us PE stalls.  Each push instruction costs ~0.65us of queue
   sequencer time -> per-chunk granularity everywhere halves effective
   bandwidth.  Ramp uses singles for kc0..5, pairs after (23 pushes).
 - ALL ramp-critical data (w0 + a combined ksub-major x block for
   m=0..6, fully-contiguous 7KB runs — an m-major layout with 1KB runs
   measurably crawls) rides the sync queue EXCLUSIVELY in consumption
   order.  Bulk loads (x7.., bias, w1..w7) sit on scalar, each gated by
   a tiny ACT-copy write into its destination sourced from a kc13 ramp
   region: the WAW dep is the only thing the compile-time Tile
   scheduler cannot hoist bulk pushes around (it DOES hoist dep-free
   instructions past a single dep-carrying gate).  Stores ride sync
   (idle after the ramp) so no load waits behind a store.
 - The PE clock starts gated at 1.2GHz and un-throttles after ~3.4us
   of sustained busy; ~2us of idle re-gates it (~1.7us cold rerun).
   28 dummy N=128 matmuls on a zeroed scratch tile start the activity
   window at the preamble end, and FILL dummies woven between early
   warm-up chunks absorb the cold-DMA gaps, so the clock reaches
   2.4GHz by ~12us and never re-throttles.
 - Warm-up interleaves m=0..6 k-chunk-outer across 7 PSUM banks (the
   8th holds the dummies), so each arriving 352KB k-chunk unlocks 7
   matmuls (1.51us) — slower than the exclusive queue delivers.
 - Bias comes pre-broadcast from host ([128,4096], one plain 1MB DMA;
   a replicating DMA costs ~8-10us of DMA-engine time).  Bias add is
   fused into the PSUM eviction on DVE (its only job).  The final
   group's eviction is split into 2x256-col strips stored on
   scalar+sync to overlap the closing HBM-write receipt.
 - Sub-region (N=256) psum starts inside one accumulation group
   mis-compute on HW (rel err 0.24) — do not re-attempt without
   understanding why.
"""

import sys

if "/opt/trn_rl_repo" not in sys.path:
    sys.path.insert(0, "/opt/trn_rl_repo")

import ml_dtypes
import numpy as np

B, S, DIN, DOUT = 4, 4096, 4096, 4096
NCORES = 8
M_TOTAL = B * S              # 16384
M_LOC = M_TOTAL // NCORES    # 2048
P = 128
M_TILES = M_LOC // P         # 16 m-tiles of 128 rows
N_TILE = 512
N_TILES = DOUT // N_TILE     # 8
K_SUB = DIN // P             # 32 k-subtiles of 128
K_CHUNKS = K_SUB // 2        # 16 DoubleRow chunks of 256
WARM_M = 7                   # m-groups interleaved during the w0 ramp
N_DUMMY = 36                 # HAM-warming dummy matmuls (N=128): they end
                             # ~11.2us, still before the earliest observed
                             # first-chunk arrival (12.0us), and on a
                             # slow-cold-DMA run (arrival up to 14.4us
                             # observed) they shrink the PE-idle window
                             # below the ~3.4us clock re-throttle threshold

_cached_nc = None


def _build():
    global _cached_nc
    if _cached_nc is not None:
        return _cached_nc

    import concourse.mybir as mybir
    import concourse.tile as tile
    from concourse import bacc

    nc = bacc.Bacc("TRN2", target_bir_lowering=False, debug=False,
                   num_devices=NCORES)

    # host-packed fp8 K-major tile blocks (see make_in_maps)
    xd01 = nc.dram_tensor("xd01", [P, K_SUB, WARM_M, P], mybir.dt.float8e5,
                          kind="ExternalInput")
    xd = nc.dram_tensor("xd", [M_TILES - WARM_M, P, K_SUB, P],
                        mybir.dt.float8e5, kind="ExternalInput")
    wd = nc.dram_tensor("wd", [N_TILES, P, K_SUB, N_TILE], mybir.dt.float8e4,
                        kind="ExternalInput")
    brep = nc.dram_tensor("brep", [P, DOUT], mybir.dt.float16,
                          kind="ExternalInput")
    out = nc.dram_tensor("out", [M_LOC, DOUT], mybir.dt.float16,
                         kind="ExternalOutput")

    with tile.TileContext(nc) as tc:
        with tc.tile_pool(name="w8p", bufs=3) as w8p, \
             tc.tile_pool(name="x8p", bufs=1) as x8p, \
             tc.tile_pool(name="outp", bufs=8) as outp, \
             tc.tile_pool(name="cst", bufs=1) as cst, \
             tc.tile_pool(name="psum", bufs=8, space="PSUM") as psump:

            # resident fp8 x: m=0..6 in one combined ksub-MAJOR block (so
            # one ramp DMA delivers a k-slice for all warm groups at once,
            # AND the transfer is fully contiguous — 7KB runs/partition;
            # the old m-major layout had 1KB runs and its packets crawled),
            # the rest as per-m tiles
            x01 = x8p.tile([P, K_SUB, WARM_M, P], mybir.dt.float8e5,
                           tag="x01", name="x01")
            x8 = {m: x8p.tile([P, K_SUB, P], mybir.dt.float8e5,
                              tag=f"x8_{m}", name=f"x8_{m}")
                  for m in range(WARM_M, M_TILES)}

            def xap(m, kc):
                if m < WARM_M:
                    return x01[:, 2 * kc:2 * kc + 2, m, :]
                return x8[m][:, 2 * kc:2 * kc + 2, :]

            w8 = {}

            # Each bulk load's destination gets a tiny ACT-copy write
            # sourced from a LATE ramp chunk of x01 (kc13): the DMA then
            # carries a WAW dependency on the gate, which the compile-time
            # Tile scheduler cannot hoist around (v6 measured dep-free
            # scalar pushes hoisted ahead of a single gating copy,
            # reintroducing the 2x ramp slowdown).
            def gate(dst2):
                nc.scalar.copy(dst2, x01[0:1, 27, WARM_M - 1, 126:128])

            def load_w(j):
                w8[j] = w8p.tile([P, K_SUB, N_TILE], mybir.dt.float8e4,
                                 tag="w8", name=f"w8_{j}")
                gate(w8[j][0:1, 31, 510:512])
                nc.scalar.dma_start(w8[j][:], wd[j, :, :, :])

            # ---- sync-queue program: strict priority order.  Emission
            # order = per-queue FIFO order; the HWDGE ring pops descriptors
            # in order, so data ARRIVES in consumption order.  Nothing else
            # rides sync until the tail, so the ramp gets the full ~350GB/s.
            # Step granularity balances two measured failure modes: a
            # chunk's matmuls gate on the completion semaphore of the
            # whole transfer that wrote it (coarse steps -> ~3us stalls),
            # while per-chunk steps double the push count and halve the
            # queue's effective bandwidth (~0.65us/push sequencer cost).
            # Singles for kc0..3 (cold window), pairs after.
            RAMP = [(2 * k, 2 * k + 2) for k in range(6)] + \
                   [(12 + 4 * s, 16 + 4 * s) for s in range(5)]
            w8[0] = w8p.tile([P, K_SUB, N_TILE], mybir.dt.float8e4,
                             tag="w8", name="w8_0")
            # A single serial sync stream beats splitting the ramp across
            # sync+scalar: the parallel variants start ~1us earlier (the
            # cold-DMA limit is per-queue) but the two queues don't
            # co-pace reliably and ~1us chunk-lag stalls eat the gain
            # (measured 462.9/464.2 vs 461.9us serial).
            first = True
            for a, b in RAMP:
                nc.sync.dma_start(w8[0][:, a:b, :], wd[0, :, a:b, :])
                if first:
                    # split the first x chunk so MM(m=0,kc=0) unblocks on
                    # 160KB instead of 352KB (cold DMA is slow)
                    nc.sync.dma_start(x01[:, a:b, 0:1, :],
                                      xd01[:, a:b, 0:1, :])
                    nc.sync.dma_start(x01[:, a:b, 1:WARM_M, :],
                                      xd01[:, a:b, 1:WARM_M, :])
                    first = False
                else:
                    nc.sync.dma_start(x01[:, a:b, :, :], xd01[:, a:b, :, :])

            # ---- scalar-queue program: all slack-tolerant bulk, each
            # transfer gated behind the ramp (see gate() above) so none of
            # it competes with the ramp for DMA bandwidth (v5 measured the
            # shared-queue ramp at half rate).  Stores ride sync (free
            # after the ramp) — never behind a pool-anti-dep-blocked push.
            gate(x8[WARM_M][0:1, 31, 126:128])
            nc.scalar.dma_start(x8[WARM_M][:], xd[0, :, :, :])
            bias_rep = cst.tile([P, DOUT], mybir.dt.float16)
            gate(bias_rep[0:1, 0:2])
            nc.scalar.dma_start(bias_rep[:], brep.ap())
            for m in range(WARM_M + 1, M_TILES):
                gate(x8[m][0:1, 31, 126:128])
                nc.scalar.dma_start(x8[m][:], xd[m - WARM_M, :, :, :])
            for j in range(1, N_TILES):
                # w3.. pushes wait on the 3-deep pool's anti-dep (column
                # j-3 finished) — head-of-line blocking is fine, nothing
                # urgent behind them on scalar
                load_w(j)

            # ---- HAM warm-up: dummy matmuls on a zeroed scratch tile keep
            # the PE activity window busy from the preamble end until the
            # first real chunk lands (~9.5us), so the 2.4GHz un-throttle
            # fires at ~10.7us instead of ~16.5us.
            dum = cst.tile([P, 2, P], mybir.dt.float8e5, name="dum")
            nc.vector.memset(dum[:], 0)
            psum = {}
            dps = psump.tile([P, P], mybir.dt.float32, tag="ps", name="ps_dum")

            def dummies(n):
                for _ in range(n):
                    nc.tensor.matmul(
                        dps[:], dum[:], dum[:], start=True, stop=True,
                        perf_mode=mybir.MatmulPerfMode.DoubleRow,
                    )

            dummies(N_DUMMY)

            def mm(j, m, kc):
                nc.tensor.matmul(
                    psum[m][:],
                    xap(m, kc),
                    w8[j][:, 2 * kc:2 * kc + 2, :],
                    start=(kc == 0),
                    stop=(kc == K_CHUNKS - 1),
                    perf_mode=mybir.MatmulPerfMode.DoubleRow,
                )

            def evict(j, m, split=False):
                if not split:
                    ob = outp.tile([P, N_TILE], mybir.dt.float16, tag="ob",
                                   name=f"ob_{j}_{m}")
                    nc.vector.tensor_add(
                        ob[:], psum[m][:],
                        bias_rep[:, j * N_TILE:(j + 1) * N_TILE])
                    nc.sync.dma_start(
                        out[m * P:(m + 1) * P,
                            j * N_TILE:(j + 1) * N_TILE], ob[:])
                    return
                # final group: halve the eviction and alternate the stores
                # across both HWDGE queues so the closing HBM-write receipt
                # overlaps the last DVE strip (4 even strips and a 384/128
                # asymmetric split both measured worse: the strips
                # serialize at ~325ns each on the DVE)
                h = N_TILE // 2
                for c in range(2):
                    eng = nc.scalar if c % 2 == 0 else nc.sync
                    ob = outp.tile([P, h], mybir.dt.float16, tag="obs",
                                   name=f"ob_{j}_{m}_{c}")
                    nc.vector.tensor_add(
                        ob[:], psum[m][:, c * h:(c + 1) * h],
                        bias_rep[:, j * N_TILE + c * h:
                                 j * N_TILE + (c + 1) * h])
                    eng.dma_start(
                        out[m * P:(m + 1) * P,
                            j * N_TILE + c * h:j * N_TILE + (c + 1) * h],
                        ob[:])

            def do_group(j, m):
                psum[m] = psump.tile([P, N_TILE], mybir.dt.float32, tag="ps",
                                     name=f"ps_{j}_{m}")
                for kc in range(K_CHUNKS):
                    mm(j, m, kc)
                evict(j, m,
                      split=(j == N_TILES - 1 and m == M_TILES - 1))

            # ---- warm-up: column 0, m=0..6 k-chunk-outer so each arriving
            # w0/x chunk unlocks WARM_M matmuls (PE consumes a 352KB chunk
            # in 1.5us warm — slower than the exclusive sync queue delivers).
            # Dummy fill between the early chunks absorbs the cold-DMA wall
            # (~1MB by 14us) without letting the PE idle long enough to
            # re-gate the clock.
            # FILL sizes are a measured optimum: enlarging them to chase
            # the residual ~300ns boundary waits costs MORE than it saves
            # because early fill dummies execute at the COLD clock rate
            # (107ns each, not the 53ns warm rate)
            FILL = {0: 16, 1: 6, 2: 4, 3: 3, 4: 2, 5: 3, 6: 2, 7: 1, 8: 2}
            for m in range(WARM_M):
                psum[m] = psump.tile([P, N_TILE], mybir.dt.float32, tag="ps",
                                     name=f"ps_0_{m}")
            for kc in range(K_CHUNKS):
                for m in range(WARM_M):
                    mm(0, m, kc)
                    if kc == 0 and m == 0:
                        dummies(FILL[0])
                dummies(FILL.get(kc + 1, 0))
            # bias_rep lands ~27us, warm-up ends ~38us: plain fused
            # evictions work (no decoupled copy needed)
            for m in range(WARM_M):
                evict(0, m)

            # ---- steady state: column-major, group-serial; w tiles were
            # all queued upfront, paced by the pool anti-deps ----
            for m in range(WARM_M, M_TILES):
                do_group(0, m)
            for j in range(1, N_TILES):
                for m in range(M_TILES):
                    do_group(j, m)

    nc.compile()
    _cached_nc = nc
    return nc


def make_in_maps(x, weight, bias):
    x = np.asarray(x)
    weight = np.asarray(weight)
    bias = np.ascontiguousarray(np.asarray(bias))
    assert x.dtype == np.float16 and weight.dtype == np.float16

    # quantize exactly as the reference does (RNE casts)
    x8 = x.astype(ml_dtypes.float8_e5m2)
    w8 = weight.astype(ml_dtypes.float8_e4m3fn)

    # weight [DOUT, DIN] -> [j, ki, ko, n]: wd[j,ki,ko,n] = w8[j*512+n,
    # ko*128+ki] (i.e. weight.T in per-tile K-major blocks)
    wd = np.ascontiguousarray(
        w8.reshape(N_TILES, N_TILE, K_SUB, P).transpose(0, 3, 2, 1))

    # bias pre-broadcast to all 128 partitions: one plain contiguous DMA
    brep = np.ascontiguousarray(np.broadcast_to(bias, (P, DOUT)))

    xf = x8.reshape(M_TOTAL, DIN)
    in_maps = []
    for c in range(NCORES):
        xc = xf[c * M_LOC:(c + 1) * M_LOC]
        # [M_LOC, DIN] -> [m-tile, ki, ko, m]: xd[t,ki,ko,m] = xc[t*128+m,
        # ko*128+ki]
        xdt = np.ascontiguousarray(
            xc.reshape(M_TILES, P, K_SUB, P).transpose(0, 3, 2, 1))
        # first WARM_M m-tiles also packed as one [ki, ko, t, m] block so
        # each ramp DMA delivers a k-slice for all warm groups at once as
        # ONE fully-contiguous transfer
        xd01 = np.ascontiguousarray(xdt[:WARM_M].transpose(1, 2, 0, 3))
        in_maps.append({"xd01": xd01, "xd": np.ascontiguousarray(xdt[WARM_M:]),
                        "wd": wd, "brep": brep})
    return in_maps


def gather_out(results):
    out = np.concatenate([r["out"] for r in results], axis=0)
    return out.reshape(B, S, DOUT)


def kernel(x, weight, bias):
    from concourse.bass_utils import run_bass_kernel_spmd

    nc = _build()
    in_maps = make_in_maps(x, weight, bias)
    res = run_bass_kernel_spmd(nc, in_maps, core_ids=list(range(NCORES)))
    return gather_out(res.results)


# revision 50
# speedup vs baseline: 1.0022x; 1.0011x over previous
"""FP8 GEMM kernel for Trainium2 (8 NeuronCores, SPMD data-parallel over tokens).

Computes: out = fp16( fp32( e5m2(x) @ e4m3(weight.T) ) + bias )
  x      [4, 4096, 4096] fp16
  weight [4096, 4096]    fp16  (out_features, in_features)
  bias   [4096]          fp16
  out    [4, 4096, 4096] fp16

Sharding: token dim (B*S = 16384) split across 8 cores (2048 rows each);
weight + bias replicated. No collectives; host concatenates the outputs.

The host quantizes both operands to fp8 (ml_dtypes RNE — bit-identical to
the reference's own jnp casts) and pre-packs them into per-tile K-major
blocks (`[tile][ki=128][ko=32][free]`), so every device load is a plain
contiguous fp8 HWDGE DMA — no in-flight cast, half the bytes of an fp16
stream.  The bias is pre-broadcast on host to [128, 4096] so the device
load is one plain 1MB DMA instead of a slow replicating DMA.

Per-core kernel, 461.9-462.8us measured vs a ~460us practical floor:
  2048 MMs x 215.8ns issue (442.0us, the fp8 DoubleRow stream rate:
  512 cols @ 2.4GHz + ~2.5ns NX) + ~2us residual ramp drift + ~13us
  cold-start (preamble ~7us + first 160KB on a cold ~40GB/s DMA) +
  ~11.3us post-MM (final evict/store/barrier ~4.2us + fixed runtime
  teardown ~7us, both inside the measured window; the preamble is
  excluded).  NOTE: the profiler drops one MATMUL record every
  10.792us (~41/run), which shows up as fake periodic 432ns "gaps" in
  the trace — they are NOT stalls (span arithmetic proves it); do not
  optimize against them.

Design (each point earned by a trace-diagnosed failure):
 - DoubleRow fp8 matmuls (K=256/instr, N=512 moving) accumulate fp32
   into PSUM; x (8MB fp8) stays resident, w n-tiles stream through a
   3-deep pool.  LDWEIGHTS (135ns) hides behind the 216ns MM stream.
 - DMA model (measured): a queue's transfers fan out as ~4KB packets
   over 16 DMA engines (~21GB/s each, ~330-380GB/s aggregate), but a
   consumer matmul gates on the completion semaphore of the WHOLE
   transfer that wrote its region -> coarse multi-chunk transfers
   cause ~3us PE stalls.  Each push instruction costs ~0.65us of queue
   sequencer time -> per-chunk granularity everywhere halves effective
   bandwidth.  Ramp uses singles for kc0..5, pairs after (23 pushes).
 - ALL ramp-critical data (w0 + a combined ksub-major x block for
   m=0..6, fully-contiguous 7KB runs — an m-major layout with 1KB runs
   measurably crawls) rides the sync queue EXCLUSIVELY in consumption
   order.  Bulk loads (x7.., bias, w1..w7) sit on scalar, each gated by
   a tiny ACT-copy write into its destination sourced from a kc13 ramp
   region: the WAW dep is the only thing the compile-time Tile
   scheduler cannot hoist bulk pushes around (it DOES hoist dep-free
   instructions past a single dep-carrying gate).  Stores ride sync
   (idle after the ramp) so no load waits behind a store.
 - The PE clock starts gated at 1.2GHz and un-throttles after ~3.4us
   of sustained busy; ~2us of idle re-gates it (~1.7us cold rerun).
   28 dummy N=128 matmuls on a zeroed scratch tile start the activity
   window at the preamble end, and FILL dummies woven between early
   warm-up chunks absorb the cold-DMA gaps, so the clock reaches
   2.4GHz by ~12us and never re-throttles.
 - Warm-up interleaves m=0..6 k-chunk-outer across 7 PSUM banks (the
   8th holds the dummies), so each arriving 352KB k-chunk unlocks 7
   matmuls (1.51us) — slower than the exclusive queue delivers.
 - Bias comes pre-broadcast from host ([128,4096], one plain 1MB DMA;
   a replicating DMA costs ~8-10us of DMA-engine time).  Bias add is
   fused into the PSUM eviction on DVE (its only job).  The final
   group's eviction is split into 2x256-col strips stored on
   scalar+sync to overlap the closing HBM-write receipt.
 - Sub-region (N=256) psum starts inside one accumulation group
   mis-compute on HW (rel err 0.24) — do not re-attempt without
   understanding why.
"""

import sys

if "/opt/trn_rl_repo" not in sys.path:
    sys.path.insert(0, "/opt/trn_rl_repo")

import ml_dtypes
import numpy as np

B, S, DIN, DOUT = 4, 4096, 4096, 4096
NCORES = 8
M_TOTAL = B * S              # 16384
M_LOC = M_TOTAL // NCORES    # 2048
P = 128
M_TILES = M_LOC // P         # 16 m-tiles of 128 rows
N_TILE = 512
N_TILES = DOUT // N_TILE     # 8
K_SUB = DIN // P             # 32 k-subtiles of 128
K_CHUNKS = K_SUB // 2        # 16 DoubleRow chunks of 256
WARM_M = 7                   # m-groups interleaved during the w0 ramp
N_DUMMY = 36                 # HAM-warming dummy matmuls (N=128): they end
                             # ~11.2us, still before the earliest observed
                             # first-chunk arrival (12.0us), and on a
                             # slow-cold-DMA run (arrival up to 14.4us
                             # observed) they shrink the PE-idle window
                             # below the ~3.4us clock re-throttle threshold

_cached_nc = None


def _build():
    global _cached_nc
    if _cached_nc is not None:
        return _cached_nc

    import concourse.mybir as mybir
    import concourse.tile as tile
    from concourse import bacc

    nc = bacc.Bacc("TRN2", target_bir_lowering=False, debug=False,
                   num_devices=NCORES)

    # host-packed fp8 K-major tile blocks (see make_in_maps)
    xd01 = nc.dram_tensor("xd01", [P, K_SUB, WARM_M, P], mybir.dt.float8e5,
                          kind="ExternalInput")
    xd = nc.dram_tensor("xd", [M_TILES - WARM_M, P, K_SUB, P],
                        mybir.dt.float8e5, kind="ExternalInput")
    wd = nc.dram_tensor("wd", [N_TILES, P, K_SUB, N_TILE], mybir.dt.float8e4,
                        kind="ExternalInput")
    brep = nc.dram_tensor("brep", [P, DOUT], mybir.dt.float16,
                          kind="ExternalInput")
    out = nc.dram_tensor("out", [M_LOC, DOUT], mybir.dt.float16,
                         kind="ExternalOutput")

    with tile.TileContext(nc) as tc:
        with tc.tile_pool(name="w8p", bufs=3) as w8p, \
             tc.tile_pool(name="x8p", bufs=1) as x8p, \
             tc.tile_pool(name="outp", bufs=8) as outp, \
             tc.tile_pool(name="cst", bufs=1) as cst, \
             tc.tile_pool(name="psum", bufs=8, space="PSUM") as psump:

            # resident fp8 x: m=0..6 in one combined ksub-MAJOR block (so
            # one ramp DMA delivers a k-slice for all warm groups at once,
            # AND the transfer is fully contiguous — 7KB runs/partition;
            # the old m-major layout had 1KB runs and its packets crawled),
            # the rest as per-m tiles
            x01 = x8p.tile([P, K_SUB, WARM_M, P], mybir.dt.float8e5,
                           tag="x01", name="x01")
            x8 = {m: x8p.tile([P, K_SUB, P], mybir.dt.float8e5,
                              tag=f"x8_{m}", name=f"x8_{m}")
                  for m in range(WARM_M, M_TILES)}

            def xap(m, kc):
                if m < WARM_M:
                    return x01[:, 2 * kc:2 * kc + 2, m, :]
                return x8[m][:, 2 * kc:2 * kc + 2, :]

            w8 = {}

            # Each bulk load's destination gets a tiny ACT-copy write
            # sourced from a LATE ramp chunk of x01 (kc13): the DMA then
            # carries a WAW dependency on the gate, which the compile-time
            # Tile scheduler cannot hoist around (v6 measured dep-free
            # scalar pushes hoisted ahead of a single gating copy,
            # reintroducing the 2x ramp slowdown).
            def gate(dst2):
                nc.scalar.copy(dst2, x01[0:1, 27, WARM_M - 1, 126:128])

            def load_w(j):
                w8[j] = w8p.tile([P, K_SUB, N_TILE], mybir.dt.float8e4,
                                 tag="w8", name=f"w8_{j}")
                gate(w8[j][0:1, 31, 510:512])
                nc.scalar.dma_start(w8[j][:], wd[j, :, :, :])

            # ---- sync-queue program: strict priority order.  Emission
            # order = per-queue FIFO order; the HWDGE ring pops descriptors
            # in order, so data ARRIVES in consumption order.  Nothing else
            # rides sync until the tail, so the ramp gets the full ~350GB/s.
            # Step granularity balances two measured failure modes: a
            # chunk's matmuls gate on the completion semaphore of the
            # whole transfer that wrote it (coarse steps -> ~3us stalls),
            # while per-chunk steps double the push count and halve the
            # queue's effective bandwidth (~0.65us/push sequencer cost).
            # Singles for kc0..3 (cold window), pairs after.
            RAMP = [(2 * k, 2 * k + 2) for k in range(6)] + \
                   [(12 + 4 * s, 16 + 4 * s) for s in range(5)]
            w8[0] = w8p.tile([P, K_SUB, N_TILE], mybir.dt.float8e4,
                             tag="w8", name="w8_0")
            # A single serial sync stream beats splitting the ramp across
            # sync+scalar: the parallel variants start ~1us earlier (the
            # cold-DMA limit is per-queue) but the two queues don't
            # co-pace reliably and ~1us chunk-lag stalls eat the gain
            # (measured 462.9/464.2 vs 461.9us serial).
            first = True
            for a, b in RAMP:
                nc.sync.dma_start(w8[0][:, a:b, :], wd[0, :, a:b, :])
                if first:
                    # split the first x chunk so MM(m=0,kc=0) unblocks on
                    # 160KB instead of 352KB (cold DMA is slow)
                    nc.sync.dma_start(x01[:, a:b, 0:1, :],
                                      xd01[:, a:b, 0:1, :])
                    nc.sync.dma_start(x01[:, a:b, 1:WARM_M, :],
                                      xd01[:, a:b, 1:WARM_M, :])
                    first = False
                else:
                    nc.sync.dma_start(x01[:, a:b, :, :], xd01[:, a:b, :, :])

            # ---- scalar-queue program: all slack-tolerant bulk, each
            # transfer gated behind the ramp (see gate() above) so none of
            # it competes with the ramp for DMA bandwidth (v5 measured the
            # shared-queue ramp at half rate).  Stores ride sync (free
            # after the ramp) — never behind a pool-anti-dep-blocked push.
            gate(x8[WARM_M][0:1, 31, 126:128])
            nc.scalar.dma_start(x8[WARM_M][:], xd[0, :, :, :])
            bias_rep = cst.tile([P, DOUT], mybir.dt.float16)
            gate(bias_rep[0:1, 0:2])
            nc.scalar.dma_start(bias_rep[:], brep.ap())
            for m in range(WARM_M + 1, M_TILES):
                gate(x8[m][0:1, 31, 126:128])
                nc.scalar.dma_start(x8[m][:], xd[m - WARM_M, :, :, :])
            for j in range(1, N_TILES):
                # w3.. pushes wait on the 3-deep pool's anti-dep (column
                # j-3 finished) — head-of-line blocking is fine, nothing
                # urgent behind them on scalar
                load_w(j)

            # ---- HAM warm-up: dummy matmuls on a zeroed scratch tile keep
            # the PE activity window busy from the preamble end until the
            # first real chunk lands (~9.5us), so the 2.4GHz un-throttle
            # fires at ~10.7us instead of ~16.5us.
            dum = cst.tile([P, 2, P], mybir.dt.float8e5, name="dum")
            nc.vector.memset(dum[:], 0)
            psum = {}
            dps = psump.tile([P, P], mybir.dt.float32, tag="ps", name="ps_dum")

            def dummies(n):
                for _ in range(n):
                    nc.tensor.matmul(
                        dps[:], dum[:], dum[:], start=True, stop=True,
                        perf_mode=mybir.MatmulPerfMode.DoubleRow,
                    )

            dummies(N_DUMMY)

            def mm(j, m, kc):
                nc.tensor.matmul(
                    psum[m][:],
                    xap(m, kc),
                    w8[j][:, 2 * kc:2 * kc + 2, :],
                    start=(kc == 0),
                    stop=(kc == K_CHUNKS - 1),
                    perf_mode=mybir.MatmulPerfMode.DoubleRow,
                )

            def evict(j, m, split=False):
                if not split:
                    ob = outp.tile([P, N_TILE], mybir.dt.float16, tag="ob",
                                   name=f"ob_{j}_{m}")
                    nc.vector.tensor_add(
                        ob[:], psum[m][:],
                        bias_rep[:, j * N_TILE:(j + 1) * N_TILE])
                    nc.sync.dma_start(
                        out[m * P:(m + 1) * P,
                            j * N_TILE:(j + 1) * N_TILE], ob[:])
                    return
                # final group: halve the eviction and alternate the stores
                # across both HWDGE queues so the closing HBM-write receipt
                # overlaps the last DVE strip (4 even strips and a 384/128
                # asymmetric split both measured worse: the strips
                # serialize at ~325ns each on the DVE)
                h = N_TILE // 2
                for c in range(2):
                    eng = nc.scalar if c % 2 == 0 else nc.sync
                    ob = outp.tile([P, h], mybir.dt.float16, tag="obs",
                                   name=f"ob_{j}_{m}_{c}")
                    nc.vector.tensor_add(
                        ob[:], psum[m][:, c * h:(c + 1) * h],
                        bias_rep[:, j * N_TILE + c * h:
                                 j * N_TILE + (c + 1) * h])
                    eng.dma_start(
                        out[m * P:(m + 1) * P,
                            j * N_TILE + c * h:j * N_TILE + (c + 1) * h],
                        ob[:])

            def do_group(j, m):
                psum[m] = psump.tile([P, N_TILE], mybir.dt.float32, tag="ps",
                                     name=f"ps_{j}_{m}")
                for kc in range(K_CHUNKS):
                    mm(j, m, kc)
                evict(j, m,
                      split=(j == N_TILES - 1 and m == M_TILES - 1))

            # ---- warm-up: column 0, m=0..6 k-chunk-outer so each arriving
            # w0/x chunk unlocks WARM_M matmuls (PE consumes a 352KB chunk
            # in 1.5us warm — slower than the exclusive sync queue delivers).
            # Dummy fill between the early chunks absorbs the cold-DMA wall
            # (~1MB by 14us) without letting the PE idle long enough to
            # re-gate the clock.
            # FILL sizes are a measured optimum: enlarging them to chase
            # the residual ~300ns boundary waits costs MORE than it saves
            # because early fill dummies execute at the COLD clock rate
            # (107ns each, not the 53ns warm rate)
            FILL = {0: 16, 1: 6, 2: 4, 3: 3, 4: 2, 5: 3, 6: 2, 7: 1, 8: 2}
            for m in range(WARM_M):
                psum[m] = psump.tile([P, N_TILE], mybir.dt.float32, tag="ps",
                                     name=f"ps_0_{m}")
            for kc in range(K_CHUNKS):
                for m in range(WARM_M):
                    mm(0, m, kc)
                    if kc == 0 and m == 0:
                        dummies(FILL[0])
                dummies(FILL.get(kc + 1, 0))
            # bias_rep lands ~27us, warm-up ends ~38us: plain fused
            # evictions work (no decoupled copy needed)
            for m in range(WARM_M):
                evict(0, m)

            # the LAST group runs as two sequential [128,256] half-groups
            # (full-tile N=256 accumulations — verified numerically
            # correct): the low half closes 1.73us before the end and its
            # eviction+store overlap the high half's matmuls, so the
            # closing drain carries one 64KB strip instead of 128KB
            def do_last_group():
                j, m = N_TILES - 1, M_TILES - 1
                NH = N_TILE // 2
                for h, (lo, hi, eng) in enumerate(
                        [(0, NH, nc.scalar), (NH, N_TILE, nc.sync)]):
                    ps = psump.tile([P, NH], mybir.dt.float32, tag="ps",
                                    name=f"ps_last_{h}")
                    for kc in range(K_CHUNKS):
                        nc.tensor.matmul(
                            ps[:], xap(m, kc),
                            w8[j][:, 2 * kc:2 * kc + 2, lo:hi],
                            start=(kc == 0), stop=(kc == K_CHUNKS - 1),
                            perf_mode=mybir.MatmulPerfMode.DoubleRow)
                    ob = outp.tile([P, NH], mybir.dt.float16, tag="obs",
                                   name=f"ob_last_{h}")
                    nc.vector.tensor_add(
                        ob[:], ps[:],
                        bias_rep[:, j * N_TILE + lo:j * N_TILE + hi])
                    eng.dma_start(
                        out[m * P:(m + 1) * P,
                            j * N_TILE + lo:j * N_TILE + hi], ob[:])

            # ---- steady state: column-major, group-serial; w tiles were
            # all queued upfront, paced by the pool anti-deps ----
            for m in range(WARM_M, M_TILES):
                do_group(0, m)
            for j in range(1, N_TILES):
                for m in range(M_TILES):
                    if j == N_TILES - 1 and m == M_TILES - 1:
                        do_last_group()
                    else:
                        do_group(j, m)

    nc.compile()
    _cached_nc = nc
    return nc


def make_in_maps(x, weight, bias):
    x = np.asarray(x)
    weight = np.asarray(weight)
    bias = np.ascontiguousarray(np.asarray(bias))
    assert x.dtype == np.float16 and weight.dtype == np.float16

    # quantize exactly as the reference does (RNE casts)
    x8 = x.astype(ml_dtypes.float8_e5m2)
    w8 = weight.astype(ml_dtypes.float8_e4m3fn)

    # weight [DOUT, DIN] -> [j, ki, ko, n]: wd[j,ki,ko,n] = w8[j*512+n,
    # ko*128+ki] (i.e. weight.T in per-tile K-major blocks)
    wd = np.ascontiguousarray(
        w8.reshape(N_TILES, N_TILE, K_SUB, P).transpose(0, 3, 2, 1))

    # bias pre-broadcast to all 128 partitions: one plain contiguous DMA
    brep = np.ascontiguousarray(np.broadcast_to(bias, (P, DOUT)))

    xf = x8.reshape(M_TOTAL, DIN)
    in_maps = []
    for c in range(NCORES):
        xc = xf[c * M_LOC:(c + 1) * M_LOC]
        # [M_LOC, DIN] -> [m-tile, ki, ko, m]: xd[t,ki,ko,m] = xc[t*128+m,
        # ko*128+ki]
        xdt = np.ascontiguousarray(
            xc.reshape(M_TILES, P, K_SUB, P).transpose(0, 3, 2, 1))
        # first WARM_M m-tiles also packed as one [ki, ko, t, m] block so
        # each ramp DMA delivers a k-slice for all warm groups at once as
        # ONE fully-contiguous transfer
        xd01 = np.ascontiguousarray(xdt[:WARM_M].transpose(1, 2, 0, 3))
        in_maps.append({"xd01": xd01, "xd": np.ascontiguousarray(xdt[WARM_M:]),
                        "wd": wd, "brep": brep})
    return in_maps


def gather_out(results):
    out = np.concatenate([r["out"] for r in results], axis=0)
    return out.reshape(B, S, DOUT)


def kernel(x, weight, bias):
    from concourse.bass_utils import run_bass_kernel_spmd

    nc = _build()
    in_maps = make_in_maps(x, weight, bias)
    res = run_bass_kernel_spmd(nc, in_maps, core_ids=list(range(NCORES)))
    return gather_out(res.results)


# revision 51
# speedup vs baseline: 1.0023x; 1.0002x over previous
"""FP8 GEMM kernel for Trainium2 (8 NeuronCores, SPMD data-parallel over tokens).

Computes: out = fp16( fp32( e5m2(x) @ e4m3(weight.T) ) + bias )
  x      [4, 4096, 4096] fp16
  weight [4096, 4096]    fp16  (out_features, in_features)
  bias   [4096]          fp16
  out    [4, 4096, 4096] fp16

Sharding: token dim (B*S = 16384) split across 8 cores (2048 rows each);
weight + bias replicated. No collectives; host concatenates the outputs.

The host quantizes both operands to fp8 (ml_dtypes RNE — bit-identical to
the reference's own jnp casts) and pre-packs them into per-tile K-major
blocks (`[tile][ki=128][ko=32][free]`), so every device load is a plain
contiguous fp8 HWDGE DMA — no in-flight cast, half the bytes of an fp16
stream.  The bias is pre-broadcast on host to [128, 4096] so the device
load is one plain 1MB DMA instead of a slow replicating DMA.

Per-core kernel, 461.9-462.8us measured vs a ~460us practical floor:
  2048 MMs x 215.8ns issue (442.0us, the fp8 DoubleRow stream rate:
  512 cols @ 2.4GHz + ~2.5ns NX) + ~2us residual ramp drift + ~13us
  cold-start (preamble ~7us + first 160KB on a cold ~40GB/s DMA) +
  ~11.3us post-MM (final evict/store/barrier ~4.2us + fixed runtime
  teardown ~7us, both inside the measured window; the preamble is
  excluded).  NOTE: the profiler drops one MATMUL record every
  10.792us (~41/run), which shows up as fake periodic 432ns "gaps" in
  the trace — they are NOT stalls (span arithmetic proves it); do not
  optimize against them.

Design (each point earned by a trace-diagnosed failure):
 - DoubleRow fp8 matmuls (K=256/instr, N=512 moving) accumulate fp32
   into PSUM; x (8MB fp8) stays resident, w n-tiles stream through a
   3-deep pool.  LDWEIGHTS (135ns) hides behind the 216ns MM stream.
 - DMA model (measured): a queue's transfers fan out as ~4KB packets
   over 16 DMA engines (~21GB/s each, ~330-380GB/s aggregate), but a
   consumer matmul gates on the completion semaphore of the WHOLE
   transfer that wrote its region -> coarse multi-chunk transfers
   cause ~3us PE stalls.  Each push instruction costs ~0.65us of queue
   sequencer time -> per-chunk granularity everywhere halves effective
   bandwidth.  Ramp uses singles for kc0..5, pairs after (23 pushes).
 - ALL ramp-critical data (w0 + a combined ksub-major x block for
   m=0..6, fully-contiguous 7KB runs — an m-major layout with 1KB runs
   measurably crawls) rides the sync queue EXCLUSIVELY in consumption
   order.  Bulk loads (x7.., bias, w1..w7) sit on scalar, each gated by
   a tiny ACT-copy write into its destination sourced from a kc13 ramp
   region: the WAW dep is the only thing the compile-time Tile
   scheduler cannot hoist bulk pushes around (it DOES hoist dep-free
   instructions past a single dep-carrying gate).  Stores ride sync
   (idle after the ramp) so no load waits behind a store.
 - The PE clock starts gated at 1.2GHz and un-throttles after ~3.4us
   of sustained busy; ~2us of idle re-gates it (~1.7us cold rerun).
   28 dummy N=128 matmuls on a zeroed scratch tile start the activity
   window at the preamble end, and FILL dummies woven between early
   warm-up chunks absorb the cold-DMA gaps, so the clock reaches
   2.4GHz by ~12us and never re-throttles.
 - Warm-up interleaves m=0..6 k-chunk-outer across 7 PSUM banks (the
   8th holds the dummies), so each arriving 352KB k-chunk unlocks 7
   matmuls (1.51us) — slower than the exclusive queue delivers.
 - Bias comes pre-broadcast from host ([128,4096], one plain 1MB DMA;
   a replicating DMA costs ~8-10us of DMA-engine time).  Bias add is
   fused into the PSUM eviction on DVE (its only job).  The final
   group runs as two sequential [128,256] half-groups so the low
   half's eviction+store overlap the high half's matmuls and the
   closing drain carries one 64KB strip (measured lowest post-MM
   tail of all variants tried; 2/4-strip splits of a single-close
   group were all worse).
 - Sub-region (N=256) psum starts inside one accumulation group
   mis-compute on HW (rel err 0.24) — do not re-attempt without
   understanding why.
"""

import sys

if "/opt/trn_rl_repo" not in sys.path:
    sys.path.insert(0, "/opt/trn_rl_repo")

import ml_dtypes
import numpy as np

B, S, DIN, DOUT = 4, 4096, 4096, 4096
NCORES = 8
M_TOTAL = B * S              # 16384
M_LOC = M_TOTAL // NCORES    # 2048
P = 128
M_TILES = M_LOC // P         # 16 m-tiles of 128 rows
N_TILE = 512
N_TILES = DOUT // N_TILE     # 8
K_SUB = DIN // P             # 32 k-subtiles of 128
K_CHUNKS = K_SUB // 2        # 16 DoubleRow chunks of 256
WARM_M = 7                   # m-groups interleaved during the w0 ramp
N_DUMMY = 36                 # HAM-warming dummy matmuls (N=128): they end
                             # ~11.2us, still before the earliest observed
                             # first-chunk arrival (12.0us), and on a
                             # slow-cold-DMA run (arrival up to 14.4us
                             # observed) they shrink the PE-idle window
                             # below the ~3.4us clock re-throttle threshold

_cached_nc = None


def _build():
    global _cached_nc
    if _cached_nc is not None:
        return _cached_nc

    import concourse.mybir as mybir
    import concourse.tile as tile
    from concourse import bacc

    nc = bacc.Bacc("TRN2", target_bir_lowering=False, debug=False,
                   num_devices=NCORES)

    # host-packed fp8 K-major tile blocks (see make_in_maps)
    xd01 = nc.dram_tensor("xd01", [P, K_SUB, WARM_M, P], mybir.dt.float8e5,
                          kind="ExternalInput")
    xd = nc.dram_tensor("xd", [M_TILES - WARM_M, P, K_SUB, P],
                        mybir.dt.float8e5, kind="ExternalInput")
    wd = nc.dram_tensor("wd", [N_TILES, P, K_SUB, N_TILE], mybir.dt.float8e4,
                        kind="ExternalInput")
    brep = nc.dram_tensor("brep", [P, DOUT], mybir.dt.float16,
                          kind="ExternalInput")
    out = nc.dram_tensor("out", [M_LOC, DOUT], mybir.dt.float16,
                         kind="ExternalOutput")

    with tile.TileContext(nc) as tc:
        with tc.tile_pool(name="w8p", bufs=3) as w8p, \
             tc.tile_pool(name="x8p", bufs=1) as x8p, \
             tc.tile_pool(name="outp", bufs=8) as outp, \
             tc.tile_pool(name="cst", bufs=1) as cst, \
             tc.tile_pool(name="psum", bufs=8, space="PSUM") as psump:

            # resident fp8 x: m=0..6 in one combined ksub-MAJOR block (so
            # one ramp DMA delivers a k-slice for all warm groups at once,
            # AND the transfer is fully contiguous — 7KB runs/partition;
            # the old m-major layout had 1KB runs and its packets crawled),
            # the rest as per-m tiles
            x01 = x8p.tile([P, K_SUB, WARM_M, P], mybir.dt.float8e5,
                           tag="x01", name="x01")
            x8 = {m: x8p.tile([P, K_SUB, P], mybir.dt.float8e5,
                              tag=f"x8_{m}", name=f"x8_{m}")
                  for m in range(WARM_M, M_TILES)}

            def xap(m, kc):
                if m < WARM_M:
                    return x01[:, 2 * kc:2 * kc + 2, m, :]
                return x8[m][:, 2 * kc:2 * kc + 2, :]

            w8 = {}

            # Each bulk load's destination gets a tiny ACT-copy write
            # sourced from a LATE ramp chunk of x01 (kc13): the DMA then
            # carries a WAW dependency on the gate, which the compile-time
            # Tile scheduler cannot hoist around (v6 measured dep-free
            # scalar pushes hoisted ahead of a single gating copy,
            # reintroducing the 2x ramp slowdown).
            def gate(dst2):
                nc.scalar.copy(dst2, x01[0:1, 27, WARM_M - 1, 126:128])

            def load_w(j):
                w8[j] = w8p.tile([P, K_SUB, N_TILE], mybir.dt.float8e4,
                                 tag="w8", name=f"w8_{j}")
                gate(w8[j][0:1, 31, 510:512])
                nc.scalar.dma_start(w8[j][:], wd[j, :, :, :])

            # ---- sync-queue program: strict priority order.  Emission
            # order = per-queue FIFO order; the HWDGE ring pops descriptors
            # in order, so data ARRIVES in consumption order.  Nothing else
            # rides sync until the tail, so the ramp gets the full ~350GB/s.
            # Step granularity balances two measured failure modes: a
            # chunk's matmuls gate on the completion semaphore of the
            # whole transfer that wrote it (coarse steps -> ~3us stalls),
            # while per-chunk steps double the push count and halve the
            # queue's effective bandwidth (~0.65us/push sequencer cost).
            # Singles for kc0..3 (cold window), pairs after.
            RAMP = [(2 * k, 2 * k + 2) for k in range(6)] + \
                   [(12 + 4 * s, 16 + 4 * s) for s in range(5)]
            w8[0] = w8p.tile([P, K_SUB, N_TILE], mybir.dt.float8e4,
                             tag="w8", name="w8_0")
            # A single serial sync stream beats splitting the ramp across
            # sync+scalar: the parallel variants start ~1us earlier (the
            # cold-DMA limit is per-queue) but the two queues don't
            # co-pace reliably and ~1us chunk-lag stalls eat the gain
            # (measured 462.9/464.2 vs 461.9us serial).
            first = True
            for a, b in RAMP:
                nc.sync.dma_start(w8[0][:, a:b, :], wd[0, :, a:b, :])
                if first:
                    # split the first x chunk so MM(m=0,kc=0) unblocks on
                    # 160KB instead of 352KB (cold DMA is slow)
                    nc.sync.dma_start(x01[:, a:b, 0:1, :],
                                      xd01[:, a:b, 0:1, :])
                    nc.sync.dma_start(x01[:, a:b, 1:WARM_M, :],
                                      xd01[:, a:b, 1:WARM_M, :])
                    first = False
                else:
                    nc.sync.dma_start(x01[:, a:b, :, :], xd01[:, a:b, :, :])

            # ---- scalar-queue program: all slack-tolerant bulk, each
            # transfer gated behind the ramp (see gate() above) so none of
            # it competes with the ramp for DMA bandwidth (v5 measured the
            # shared-queue ramp at half rate).  Stores ride sync (free
            # after the ramp) — never behind a pool-anti-dep-blocked push.
            gate(x8[WARM_M][0:1, 31, 126:128])
            nc.scalar.dma_start(x8[WARM_M][:], xd[0, :, :, :])
            bias_rep = cst.tile([P, DOUT], mybir.dt.float16)
            gate(bias_rep[0:1, 0:2])
            nc.scalar.dma_start(bias_rep[:], brep.ap())
            for m in range(WARM_M + 1, M_TILES):
                gate(x8[m][0:1, 31, 126:128])
                nc.scalar.dma_start(x8[m][:], xd[m - WARM_M, :, :, :])
            for j in range(1, N_TILES):
                # w3.. pushes wait on the 3-deep pool's anti-dep (column
                # j-3 finished) — head-of-line blocking is fine, nothing
                # urgent behind them on scalar
                load_w(j)

            # ---- HAM warm-up: dummy matmuls on a zeroed scratch tile keep
            # the PE activity window busy from the preamble end until the
            # first real chunk lands (~9.5us), so the 2.4GHz un-throttle
            # fires at ~10.7us instead of ~16.5us.
            dum = cst.tile([P, 2, P], mybir.dt.float8e5, name="dum")
            nc.vector.memset(dum[:], 0)
            psum = {}
            dps = psump.tile([P, P], mybir.dt.float32, tag="ps", name="ps_dum")

            def dummies(n):
                for _ in range(n):
                    nc.tensor.matmul(
                        dps[:], dum[:], dum[:], start=True, stop=True,
                        perf_mode=mybir.MatmulPerfMode.DoubleRow,
                    )

            dummies(N_DUMMY)

            def mm(j, m, kc):
                nc.tensor.matmul(
                    psum[m][:],
                    xap(m, kc),
                    w8[j][:, 2 * kc:2 * kc + 2, :],
                    start=(kc == 0),
                    stop=(kc == K_CHUNKS - 1),
                    perf_mode=mybir.MatmulPerfMode.DoubleRow,
                )

            def evict(j, m, split=False):
                if not split:
                    ob = outp.tile([P, N_TILE], mybir.dt.float16, tag="ob",
                                   name=f"ob_{j}_{m}")
                    nc.vector.tensor_add(
                        ob[:], psum[m][:],
                        bias_rep[:, j * N_TILE:(j + 1) * N_TILE])
                    nc.sync.dma_start(
                        out[m * P:(m + 1) * P,
                            j * N_TILE:(j + 1) * N_TILE], ob[:])
                    return
                # final group: halve the eviction and alternate the stores
                # across both HWDGE queues so the closing HBM-write receipt
                # overlaps the last DVE strip (4 even strips and a 384/128
                # asymmetric split both measured worse: the strips
                # serialize at ~325ns each on the DVE)
                h = N_TILE // 2
                for c in range(2):
                    eng = nc.scalar if c % 2 == 0 else nc.sync
                    ob = outp.tile([P, h], mybir.dt.float16, tag="obs",
                                   name=f"ob_{j}_{m}_{c}")
                    nc.vector.tensor_add(
                        ob[:], psum[m][:, c * h:(c + 1) * h],
                        bias_rep[:, j * N_TILE + c * h:
                                 j * N_TILE + (c + 1) * h])
                    eng.dma_start(
                        out[m * P:(m + 1) * P,
                            j * N_TILE + c * h:j * N_TILE + (c + 1) * h],
                        ob[:])

            def do_group(j, m):
                psum[m] = psump.tile([P, N_TILE], mybir.dt.float32, tag="ps",
                                     name=f"ps_{j}_{m}")
                for kc in range(K_CHUNKS):
                    mm(j, m, kc)
                evict(j, m,
                      split=(j == N_TILES - 1 and m == M_TILES - 1))

            # ---- warm-up: column 0, m=0..6 k-chunk-outer so each arriving
            # w0/x chunk unlocks WARM_M matmuls (PE consumes a 352KB chunk
            # in 1.5us warm — slower than the exclusive sync queue delivers).
            # Dummy fill between the early chunks absorbs the cold-DMA wall
            # (~1MB by 14us) without letting the PE idle long enough to
            # re-gate the clock.
            # FILL sizes are a measured optimum: enlarging them to chase
            # the residual ~300ns boundary waits costs MORE than it saves
            # because early fill dummies execute at the COLD clock rate
            # (107ns each, not the 53ns warm rate)
            FILL = {0: 16, 1: 6, 2: 4, 3: 3, 4: 2, 5: 3, 6: 2, 7: 1, 8: 2}
            for m in range(WARM_M):
                psum[m] = psump.tile([P, N_TILE], mybir.dt.float32, tag="ps",
                                     name=f"ps_0_{m}")
            for kc in range(K_CHUNKS):
                for m in range(WARM_M):
                    mm(0, m, kc)
                    if kc == 0 and m == 0:
                        dummies(FILL[0])
                dummies(FILL.get(kc + 1, 0))
            # bias_rep lands ~27us, warm-up ends ~38us: plain fused
            # evictions work (no decoupled copy needed)
            for m in range(WARM_M):
                evict(0, m)

            # the LAST group runs as two sequential [128,256] half-groups
            # (full-tile N=256 accumulations — verified numerically
            # correct): the low half closes 1.73us before the end and its
            # eviction+store overlap the high half's matmuls, so the
            # closing drain carries one 64KB strip instead of 128KB
            def do_last_group():
                j, m = N_TILES - 1, M_TILES - 1
                NH = N_TILE // 2
                for h, (lo, hi, eng) in enumerate(
                        [(0, NH, nc.scalar), (NH, N_TILE, nc.sync)]):
                    ps = psump.tile([P, NH], mybir.dt.float32, tag="ps",
                                    name=f"ps_last_{h}")
                    for kc in range(K_CHUNKS):
                        nc.tensor.matmul(
                            ps[:], xap(m, kc),
                            w8[j][:, 2 * kc:2 * kc + 2, lo:hi],
                            start=(kc == 0), stop=(kc == K_CHUNKS - 1),
                            perf_mode=mybir.MatmulPerfMode.DoubleRow)
                    ob = outp.tile([P, NH], mybir.dt.float16, tag="obs",
                                   name=f"ob_last_{h}")
                    nc.vector.tensor_add(
                        ob[:], ps[:],
                        bias_rep[:, j * N_TILE + lo:j * N_TILE + hi])
                    eng.dma_start(
                        out[m * P:(m + 1) * P,
                            j * N_TILE + lo:j * N_TILE + hi], ob[:])

            # ---- steady state: column-major, group-serial; w tiles were
            # all queued upfront, paced by the pool anti-deps ----
            for m in range(WARM_M, M_TILES):
                do_group(0, m)
            for j in range(1, N_TILES):
                for m in range(M_TILES):
                    if j == N_TILES - 1 and m == M_TILES - 1:
                        do_last_group()
                    else:
                        do_group(j, m)

    nc.compile()
    _cached_nc = nc
    return nc


def make_in_maps(x, weight, bias):
    x = np.asarray(x)
    weight = np.asarray(weight)
    bias = np.ascontiguousarray(np.asarray(bias))
    assert x.dtype == np.float16 and weight.dtype == np.float16

    # quantize exactly as the reference does (RNE casts)
    x8 = x.astype(ml_dtypes.float8_e5m2)
    w8 = weight.astype(ml_dtypes.float8_e4m3fn)

    # weight [DOUT, DIN] -> [j, ki, ko, n]: wd[j,ki,ko,n] = w8[j*512+n,
    # ko*128+ki] (i.e. weight.T in per-tile K-major blocks)
    wd = np.ascontiguousarray(
        w8.reshape(N_TILES, N_TILE, K_SUB, P).transpose(0, 3, 2, 1))

    # bias pre-broadcast to all 128 partitions: one plain contiguous DMA
    brep = np.ascontiguousarray(np.broadcast_to(bias, (P, DOUT)))

    xf = x8.reshape(M_TOTAL, DIN)
    in_maps = []
    for c in range(NCORES):
        xc = xf[c * M_LOC:(c + 1) * M_LOC]
        # [M_LOC, DIN] -> [m-tile, ki, ko, m]: xd[t,ki,ko,m] = xc[t*128+m,
        # ko*128+ki]
        xdt = np.ascontiguousarray(
            xc.reshape(M_TILES, P, K_SUB, P).transpose(0, 3, 2, 1))
        # first WARM_M m-tiles also packed as one [ki, ko, t, m] block so
        # each ramp DMA delivers a k-slice for all warm groups at once as
        # ONE fully-contiguous transfer
        xd01 = np.ascontiguousarray(xdt[:WARM_M].transpose(1, 2, 0, 3))
        in_maps.append({"xd01": xd01, "xd": np.ascontiguousarray(xdt[WARM_M:]),
                        "wd": wd, "brep": brep})
    return in_maps


def gather_out(results):
    out = np.concatenate([r["out"] for r in results], axis=0)
    return out.reshape(B, S, DOUT)


def kernel(x, weight, bias):
    from concourse.bass_utils import run_bass_kernel_spmd

    nc = _build()
    in_maps = make_in_maps(x, weight, bias)
    res = run_bass_kernel_spmd(nc, in_maps, core_ids=list(range(NCORES)))
    return gather_out(res.results)
